# revision 1
# baseline (speedup 1.0000x reference)
"""Batch-softmax attention kernel for Trainium2 (8 NeuronCores), v2.

Problem: out[b,h,i,v] = sum_j softmax_over_b(QK^T/sqrt(H))[b,h,i,j] * V[b,h,j,v]
with B=4, H=8, S=2048, D=64.  Softmax is over the BATCH axis (dim=0).

Sharding: one head per NeuronCore (H=8 across 8 cores); batch softmax is
purely local.

v2 "difference softmax" dataflow (vs v1 which exp'd all 4 batches):
  W_b = E'_b * r',  E'_0 = 1,  E'_b = exp(scale*(s_b - s_0)) for b=1..3,
  r' = 1/(1 + E'_1 + E'_2 + E'_3).
  - PE computes d_b = s_b - s_0 DIRECTLY: stationary [k_b ; -k_0] (128 rows,
    full contraction -> 100% array util), moving [q_b ; q_0].  192 QK matmuls
    instead of 256, and only THREE exp evacuations instead of four.
  - ACT: exp of 3 diffs per j-tile in one instr (PSUM [128,1536] -> bf16 SBUF).
  - DVE: t = E'2+E'3 (2x), r' = ADD_RECIP_1P (custom: recip(1+in0+in1), 1x),
    W1 = E'1*r', W2 = E'2*r' (2x).  W_0 := r' needs NO multiply.
  - W3 = E'3*r' on GPSIMD (otherwise idle) to keep DVE ~93us.
  - WV: V bf16 stationary, W bf16 moving, col-paired tile_position like v1.
Engine budgets/core: ACT ~95us, DVE ~93us, PE ~100us, GPSIMD ~71us.
"""

import math
import os
import sys

import numpy as np

sys.path.insert(0, "/opt/trn_rl_repo")
os.environ.setdefault("MYCRO_LOCAL_CACHE", "1")

B, H, S, D = 4, 8, 2048, 64
N_CORES = 8
SCALE = 1.0 / math.sqrt(H)  # NOTE: reference scales by sqrt(num_heads)

IC = 4          # i-chunks of 512 columns
ICW = S // IC   # 512
JT = S // 128   # 16 j-tiles of 128 rows
NPAIR = JT // 2
HEADJ = 4       # j-tiles covered by the starter K tiles
W3_ENGINE = os.environ.get("K_W3", "gpsimd")
EPOOL_BUFS = int(os.environ.get("K_EPOOL", "3"))

_CACHED_NC = None
_ADD_RECIP_1P = None


def _register_add_recip_1p():
    """Custom DVE op: out = recip_approx(1 + in0 + in1), 1 Newton step."""
    global _ADD_RECIP_1P
    if _ADD_RECIP_1P is not None:
        return _ADD_RECIP_1P
    import numpy as np_
    import concourse.dve_ops as dvo
    from concourse.dve_spec import AluOp, Bin, C0, C1, One, Spec, Src0, Src1, lower
    from concourse.dve_uop import DveOpSpec

    _x = Bin(AluOp.ADD, Bin(AluOp.ADD, Src0, Src1), One)
    _nx = Bin(AluOp.BITWISE_NOT, _x, _x)
    _y0 = _nx * C0
    _body = _y0 * (C1 - _x * _y0)

    def _ref(in0, in1, s0, s1, imm2):
        x = (in0 + in1 + np_.float32(1.0)).astype(np_.float32)
        nx = (~x.view(np_.int32)).view(np_.float32)
        y0 = nx * np_.float32(s0)
        return y0 * (np_.float32(s1) - x * y0)

    name = "ADD_RECIP_1P_ANT"
    op = dvo.DveOp(name, Spec(body=_body, reference=_ref), subdim=False,
                   uops_sha={})
    dvo.OPS.append(op)
    dvo.CUSTOM_DVE_SPECS[name] = op.spec
    dvo._SUB_OPCODE_FOR_NAME[name] = dvo._CUSTOM_DVE_ROW_BASE + len(dvo.OPS) - 1
    assert dvo._SUB_OPCODE_FOR_NAME[name] < 0x20
    shas = {}
    for ver in ("v3", "v4"):
        s = DveOpSpec(name=name, opcode=dvo.get_dve_sub_opcode(name),
                      uops=lower(op.spec, ver=ver), rd1_en=True)
        shas[ver] = s.sha(ver)
    object.__setattr__(op, "uops_sha", shas)
    _ADD_RECIP_1P = op
    return op


def _build_nc():
    from concourse import bacc, tile
    from concourse.bass import mybir
    from concourse.dve_ops import RECIP_APPROX_FAST_CONSTS

    add_recip_1p = _register_add_recip_1p()

    f32 = mybir.dt.float32
    f16 = mybir.dt.float16
    bf16 = mybir.dt.bfloat16
    Exp = mybir.ActivationFunctionType.Exp
    rc = RECIP_APPROX_FAST_CONSTS

    nc = bacc.Bacc("TRN2", target_bir_lowering=False, debug=False)

    # inputs (per core = one head); kd rows 64:128 hold -k0^T so the matmul
    # computes s_b - s_0 with full-128 contraction
    kh_in = nc.dram_tensor("kh", [128, 3, HEADJ * 128], f16, kind="ExternalInput").ap()
    kr_in = nc.dram_tensor("kr", [128, 3, S - HEADJ * 128], f16, kind="ExternalInput").ap()
    qh_in = nc.dram_tensor("qh", [128, 3, ICW], f16, kind="ExternalInput").ap()
    qr_in = nc.dram_tensor("qr", [128, 3, S - ICW], f16, kind="ExternalInput").ap()
    vh_in = nc.dram_tensor("vh", [128, B, 2, D], bf16, kind="ExternalInput").ap()
    vr_in = nc.dram_tensor("vr", [128, B, JT - 2, D], bf16, kind="ExternalInput").ap()
    out_d = nc.dram_tensor("out", [B, D, S], f32, kind="ExternalOutput").ap()

    with tile.TileContext(nc) as tc:
        with (
            tc.tile_pool(name="wts", bufs=1) as wpool,
            tc.tile_pool(name="ep", bufs=3) as epool,
            tc.tile_pool(name="tp", bufs=2) as tpool,
            tc.tile_pool(name="rp", bufs=3) as rpool,
            tc.tile_pool(name="wp", bufs=2) as wpool2,
            tc.tile_pool(name="osb", bufs=2) as opool,
            tc.tile_pool(name="ps", bufs=2, space="PSUM") as psp,
            tc.tile_pool(name="po", bufs=1, space="PSUM") as pop,
        ):
            KHT = wpool.tile([128, 3, HEADJ * 128], f16, tag="kht")
            QHT = wpool.tile([128, 3, ICW], f16, tag="qht")
            VHT = wpool.tile([128, B, 2, D], bf16, tag="vht")
            KRT = wpool.tile([128, 3, S - HEADJ * 128], f16, tag="krt")
            QRT = wpool.tile([128, 3, S - ICW], f16, tag="qrt")
            VRT = wpool.tile([128, B, JT - 2, D], bf16, tag="vrt")
            # small head DMAs first so compute ramps immediately
            nc.sync.dma_start(out=KHT[:], in_=kh_in)
            nc.sync.dma_start(out=QHT[:], in_=qh_in)
            nc.sync.dma_start(out=VHT[:], in_=vh_in)
            nc.sync.dma_start(out=VRT[:], in_=vr_in)
            nc.sync.dma_start(out=KRT[:], in_=kr_in)
            nc.sync.dma_start(out=QRT[:], in_=qr_in)

            def k_slice(bi, jt):
                if jt < HEADJ:
                    return KHT[:, bi, jt * 128:(jt + 1) * 128]
                return KRT[:, bi, (jt - HEADJ) * 128:(jt - HEADJ + 1) * 128]

            def q_slice(bi, c):
                if c == 0:
                    return QHT[:, bi, :]
                return QRT[:, bi, (c - 1) * ICW:c * ICW]

            def v_slice(b, jt):
                if jt < 2:
                    return VHT[:, b, jt, :]
                return VRT[:, b, jt - 2, :]

            # j-group schedule: small groups at the pipeline ramp (first
            # chunk) and drain (last chunk) shorten the serial critical path
            def schedule(c):
                if c == 0:
                    return [1, 1, 2, 4, 4, 4]
                if c == IC - 1:
                    return [4, 4, 4, 2, 1, 1]
                return [4, 4, 4, 4]

            for c in range(IC):
                out01 = pop.tile([128, ICW], f32, tag="o01")
                out23 = pop.tile([128, ICW], f32, tag="o23")

                def emit_wv(j0, JG, R, W):
                    for u in range(JG):
                        jt = j0 + u
                        for b, (po_t, base) in enumerate(
                            [(out01, 0), (out01, 64), (out23, 0), (out23, 64)]
                        ):
                            rhs = R[:, u, :] if b == 0 else W[:, u, b - 1, :]
                            nc.tensor.matmul(
                                po_t[base:base + 64, :], v_slice(b, jt), rhs,
                                start=(jt == 0), stop=(jt == JT - 1),
                                tile_position=(0, base), skip_group_check=True)

                j0 = 0
                for JG in schedule(c):
                    E = epool.tile([128, JG, 3, ICW], bf16, tag="E")
                    for u in range(JG):
                        jt = j0 + u
                        SP = psp.tile([128, 3, ICW], f32, tag="sp")
                        for bi in range(3):
                            nc.tensor.matmul(
                                SP[:, bi, :], k_slice(bi, jt), q_slice(bi, c),
                                start=True, stop=True)
                        nc.scalar.activation(E[:, u], SP[:], Exp, scale=SCALE)
                    T = tpool.tile([128, JG, ICW], bf16, tag="T")
                    nc.vector.tensor_add(T[:], E[:, :, 1, :].opt(),
                                         E[:, :, 2, :].opt())
                    R = rpool.tile([128, JG, ICW], bf16, tag="R")
                    nc.vector._custom_dve(
                        add_recip_1p, out=R[:], in0=E[:, :, 0, :].opt(),
                        in1=T[:], s0=rc["s0"], s1=rc["s1"])
                    W = wpool2.tile([128, JG, 3, ICW], bf16, tag="W")
                    # all three W's in ONE 2x op (GPSIMD contends for the
                    # DVE SBUF port, so it LOSES time overall -- keep on DVE):
                    # contiguous dst, r broadcast over the b' axis (0-stride)
                    r3b = R[:].unsqueeze(2).broadcast_to([128, JG, 3, ICW])
                    nc.vector.tensor_mul(W[:], E[:], r3b)
                    emit_wv(j0, JG, R, W)
                    j0 += JG

                isl = slice(c * ICW, (c + 1) * ICW)
                OSB01 = opool.tile([128, ICW], f32, tag="osb01")
                OSB23 = opool.tile([128, ICW], f32, tag="osb23")
                last = c == IC - 1
                eng23 = nc.scalar if last else nc.sync
                nc.scalar.copy(OSB01[:], out01[:])
                nc.sync.dma_start(out=out_d[0, :, isl], in_=OSB01[0:64, :])
                nc.sync.dma_start(out=out_d[1, :, isl], in_=OSB01[64:128, :])
                nc.scalar.copy(OSB23[:], out23[:])
                eng23.dma_start(out=out_d[2, :, isl], in_=OSB23[0:64, :])
                eng23.dma_start(out=out_d[3, :, isl], in_=OSB23[64:128, :])

    nc.compile()
    return nc


def _get_nc():
    global _CACHED_NC
    if _CACHED_NC is None:
        _CACHED_NC = _build_nc()
    return _CACHED_NC


def _make_in_maps(query, key, value):
    import ml_dtypes
    bf16 = ml_dtypes.bfloat16
    q16 = query.astype(np.float16)
    k16 = key.astype(np.float16)
    vbf = value.astype(bf16)
    in_maps = []
    hj = HEADJ * 128
    for h in range(H):
        kT = [np.ascontiguousarray(k16[b, h].T) for b in range(B)]  # [64, S]
        qT = [np.ascontiguousarray(q16[b, h].T) for b in range(B)]
        nk0 = -kT[0]
        kd = [np.concatenate([kT[b], nk0], axis=0) for b in (1, 2, 3)]
        qd = [np.concatenate([qT[b], qT[0]], axis=0) for b in (1, 2, 3)]
        kh = np.stack([x[:, :hj] for x in kd], axis=1)
        kr = np.stack([x[:, hj:] for x in kd], axis=1)
        qh = np.stack([x[:, :ICW] for x in qd], axis=1)
        qr = np.stack([x[:, ICW:] for x in qd], axis=1)
        # V: [128 j-in-tile, B, JT, D]
        vv = np.stack([vbf[b, h].reshape(JT, 128, D).transpose(1, 0, 2)
                       for b in range(B)], axis=1)
        im = {
            "kh": np.ascontiguousarray(kh),
            "kr": np.ascontiguousarray(kr),
            "qh": np.ascontiguousarray(qh),
            "qr": np.ascontiguousarray(qr),
            "vh": np.ascontiguousarray(vv[:, :, 0:2, :]),
            "vr": np.ascontiguousarray(vv[:, :, 2:, :]),
        }
        in_maps.append(im)
    return in_maps


def _assemble(results):
    out = np.empty((B, H, S, D), np.float32)
    for h in range(H):
        out[:, h] = results[h]["out"].transpose(0, 2, 1)  # [B,D,S] -> [B,S,D]
    return out


def _install_profile_hook():
    """Provide antenv.axon_hooks with a ctypes NTFF profile hook so that
    run_bass_kernel_spmd(trace=True) works under axon in this container."""
    import contextlib
    import ctypes
    import types

    try:
        from antenv.axon_hooks import get_axon_ntff_profile_hook  # noqa: F401
        return  # already present
    except ImportError:
        pass

    so_path = "/opt/axon/libaxon_pjrt.so"
    lib = ctypes.CDLL(so_path)
    if not hasattr(lib, "axon_start_nrt_profile"):
        return
    lib.axon_start_nrt_profile.argtypes = [
        ctypes.POINTER(ctypes.c_int64), ctypes.c_size_t]
    lib.axon_start_nrt_profile.restype = ctypes.c_int64
    lib.axon_stop_nrt_profile.argtypes = [ctypes.c_char_p]
    lib.axon_stop_nrt_profile.restype = ctypes.c_int64

    @contextlib.contextmanager
    def _hook(output_dir, device_ids):
        import jax
        jax.devices()
        if device_ids:
            ids = (ctypes.c_int64 * len(device_ids))(*device_ids)
            rc = lib.axon_start_nrt_profile(ids, len(device_ids))
        else:
            rc = lib.axon_start_nrt_profile(None, 0)
        if rc != 0:
            raise RuntimeError(f"axon_start_nrt_profile rc={rc}")
        try:
            yield
        finally:
            n = lib.axon_stop_nrt_profile(str(output_dir).encode())
            print(f"ntff profile: {n} file(s) written to {output_dir}")

    mod = types.ModuleType("antenv.axon_hooks")
    mod.get_axon_ntff_profile_hook = lambda: _hook
    mod.set_axon_ntff_profile_hook = lambda h: None
    sys.modules["antenv.axon_hooks"] = mod


def run(query, key, value, trace=False):
    """Run the distributed kernel; returns (output, exec_time_ns or None)."""
    from concourse.bass_utils import run_bass_kernel_spmd

    if trace:
        _install_profile_hook()
    nc = _get_nc()
    in_maps = _make_in_maps(query, key, value)
    res = run_bass_kernel_spmd(nc, in_maps, core_ids=list(range(N_CORES)),
                               trace=trace)
    return _assemble(res.results), res.exec_time_ns


def kernel(query, key, value):
    out, _ = run(query, key, value, trace=False)
    return out

